# revision 1
# baseline (speedup 1.0000x reference)
"""Bahdanau additive attention on TRN2 — separable-Fourier Bass/Tile kernel.

Problem: nn_AttentionLayer_11055245820581
  e[b,y,x] = softmax_x( sum_e V[e] * tanh(Ws[b,x,e] + Uh[b,y,e]) )
  c[b,y,:] = sum_x e[b,y,x] * enc[b,x,:]
with Ws = enc @ W_a, Uh = dec @ U_a.

Sharding: data-parallel over batch B=8 across the 8 NeuronCores.

Instead of materializing the Ty*Tx*E tanh cube (16.7M elements, ~110us on
ACT), expand tanh in a sine series fit on the data range |z| <= 7:

  tanh(z) ~= sum_{m=1..M} c_m sin(m*w*z),   w = pi/L

and use sin(mw(a+b)) = sin(mwa)cos(mwb) + cos(mwa)sin(mwb), which turns the
V-weighted e-contraction into 2M rank-E fp16 matmuls on the PE:

  logitT[x,y] = sum_m  (V c_m sin_m(Ws))^T_e-contract cos_m(Uh)
              +        (V c_m cos_m(Ws))^T_e-contract sin_m(Uh)

Factor families sin_m/cos_m for BOTH sides live in one combined
[P, side, fam, ec, 256] fp16 tile per mode and advance by a single pair of
[128, 2048] DVE tensor_tensor ops per mode (Chebyshev:
s_m = 2cos(wz) s_{m-1} - s_{m-2}; the 2cos multiplier is a half-size
[t2cW|t2cU] tile read through a step-0 broadcast AP). Bases come from ACT
Sin full-angle ops: with w = pi/7.7 and |z| <= 3.82, both wz and
pi/2 - wz stay inside ACT Sin's [-pi,pi] range, so sin(wz)/cos(wz) are
emitted directly -- the U-side pair straight into the mode-1 family
slots. V is folded into the Ws-side mode-1 copies (per-partition
scalars); c_m is one ACT Copy-with-scale per mode on the Uh side
(parallel to the DVE recurrence, no act-table reload since Copy is in
every set). Softmax epilogue in the transposed layout: one ACT Exp from
PSUM (table prefetched during mode M via a pinned dummy op), denominator
via ones-matmul, context matmul with fp16 enc, PE transposes + ACT
Identity-scales for the attention weights.

HW-measured pitfalls baked into this code (TRN2):
 - multi-free-dim DVE APs fall off the fast path (5.9us vs 0.8us for the
   same 2048 elems): always _flat() DVE operands.
 - gpsimd tensor_scalar is Q7-emulated (~12us per [128,1024] op) and its
   SBUF traffic stalls concurrent DVE ops 3-14x: gpsimd only does
   dma/memset/tensor_copy here.
 - explicit start/stop accumulation bits across interleaved PSUM groups
   misassociate: memset once + start=False/stop=False everywhere.
 - ACT table loads cost 1283ns; Sin and Exp live in different sets, so
   the swap is prefetched mid-mode-loop with an input-pinned dummy.
"""

import os

# Defensive: start from clean NeuronCore state if a previous process left
# the device wedged (observed sporadic NaN/garbage after NRT_EXEC_UNIT
# errors in long sessions). Set before the runtime initializes.
os.environ.setdefault("NEURON_RT_RESET_CORES", "1")

import numpy as np
from contextlib import ExitStack

import concourse.bass as bass
import concourse.bacc as bacc
import concourse.tile as tile
from concourse import mybir
from concourse.bass_utils import run_bass_kernel_spmd

B, Tx, Ty, E, D = 8, 256, 256, 256, 256
P = 128
NCORES = 8
F32 = mybir.dt.float32
F16 = mybir.dt.float16
SIN = mybir.ActivationFunctionType.Sin
EXP = mybir.ActivationFunctionType.Exp
MULT = mybir.AluOpType.mult
ADD = mybir.AluOpType.add
SUB = mybir.AluOpType.subtract

EC = E // P      # 2 e-chunks
XC = Tx // P     # 2 x-chunks
YC = Ty // P     # 2 y-halves
DC = D // P      # 2 d-chunks

# Sine-series fit of tanh on |z|<=7 (Gaussian-weighted LSQ, L=7.7, M=7).
M_MODES = 7
L_PER = 7.7
OMEGA = float(np.pi / L_PER)
COEF = [1.2210204278736967, -0.05379368613642803, 0.3062699531823439,
        -0.05713739755626698, 0.11568715986802931, -0.043240949058180794,
        0.04315768634767346]

_NC = None
LAST_RESULTS = None

_KEEP0 = frozenset({0})


def _flat(ap):
    """Collapse contiguous free dims: multi-free-dim APs fall off the DVE
    fast path on HW (measured 5.9us vs 0.8us for the same 2048 elems)."""
    return ap.opt(_KEEP0)


def _build_body(tc, ctx, enc_d, dec_d, W_d, U_d, V_d, c_d, e_d):
    nc = tc.nc
    from concourse.masks import make_identity

    consts = ctx.enter_context(tc.tile_pool(name="consts", bufs=1))
    tmps = ctx.enter_context(tc.tile_pool(name="tmps", bufs=2))
    psA = ctx.enter_context(tc.tile_pool(name="psA", bufs=1, space="PSUM"))
    pieces = ctx.enter_context(tc.tile_pool(name="pieces", bufs=4,
                                            space="PSUM"))

    # ---- input DMA first, spread across issue engines ----
    # U-side tensors (dec, U, V) first: they gate the DVE factor chain.
    dec_sb = consts.tile([P, YC, D], F32)
    V_sb = consts.tile([P, EC], F32)
    enc_sb = consts.tile([P, XC, E], F32)
    W_sb = consts.tile([P, EC, E], F32)
    U_sb = consts.tile([P, DC, E], F32)
    # identity build leads the gpsimd queue (it gates all PE transposes)
    ident = consts.tile([P, P], F32)
    from concourse.masks import make_identity as _mkid
    _mkid(nc, ident)
    ident16 = consts.tile([P, P], F16)
    nc.gpsimd.tensor_copy(ident16[:], ident[:])
    nc.sync.dma_start(out=dec_sb[:],
                      in_=dec_d.rearrange("(c p) e -> p c e", c=YC))
    nc.sync.dma_start(out=enc_sb[:],
                      in_=enc_d.rearrange("(c p) e -> p c e", c=XC))
    nc.sync.dma_start(out=V_sb[:],
                      in_=V_d.rearrange("(c p) o -> p (c o)", c=EC))
    nc.scalar.dma_start(out=U_sb[:],
                        in_=U_d.rearrange("(c p) e -> p c e", c=DC))
    nc.scalar.dma_start(out=W_sb[:],
                        in_=W_d.rearrange("(c p) e -> p c e", c=EC))

    # ---- warmups: Sin table load + PE clock ramp (no input deps) ----
    ones_sb = consts.tile([P, 1], F32)
    nc.vector.memset(ones_sb[:], 1.0)
    halfpi_sb = consts.tile([P, 1], F32)
    nc.vector.memset(halfpi_sb[:], float(np.pi / 2))
    warm_sb = consts.tile([P, 1], F32)
    nc.scalar.activation(out=warm_sb[:], in_=ones_sb[:], func=SIN, scale=0.1)
    pe_warm = consts.tile([P, 256], F16)
    nc.vector.memset(pe_warm[:], 1.0)
    for r in range(4):
        warm_ps = pieces.tile([P, 512], F32, tag="piece", name=f"warm{r}")
        nc.tensor.matmul(out=warm_ps[:, :256], lhsT=pe_warm[:, :P],
                         rhs=pe_warm[:], start=True, stop=True,
                         skip_group_check=True)

    ones16 = consts.tile([P, 1], F16)
    nc.vector.memset(ones16[:], 1.0)

    # logit accumulator [x, (xc), y]: zeroed once, matmuls then accumulate
    # with start=False/stop=False (explicit start/stop bits on interleaved
    # groups misassociate).
    logit_ps = psA.tile([P, XC, Ty], F32)
    nc.vector.memset(logit_ps[:], 0.0)

    # ---- fp16 casts: U16 on GPSIMD; W16/enc16 on ACT (idle until bases) ----
    enc16 = consts.tile([P, XC, E], F16)    # context-matmul rhs
    W16 = consts.tile([P, EC, E], F16)
    U16 = consts.tile([P, DC, E], F16)
    for i in range(DC):
        nc.gpsimd.tensor_copy(U16[:, i, :], U_sb[:, i, :])
    for i in range(EC):
        nc.scalar.copy(W16[:, i, :], W_sb[:, i, :])
    for i in range(XC):
        nc.scalar.copy(enc16[:, i, :], enc_sb[:, i, :])

    # ---- fp32 PE transposes straight from staging; evacs cast to fp16 ----
    decT16 = consts.tile([P, DC, Ty], F16)  # [d, (dc), y]
    encT16 = consts.tile([P, EC, Tx], F16)  # [e, (ec), x]
    for i in range(YC):
        for j in range(DC):
            pt = pieces.tile([P, 512], F32, tag="piece", name=f"ptD{i}{j}")
            nc.tensor.transpose(out=pt[:, :P],
                                in_=dec_sb[:, i, j * P:(j + 1) * P],
                                identity=ident[:])
            nc.vector.tensor_copy(decT16[:, j, i * P:(i + 1) * P], pt[:, :P])
    for i in range(XC):
        for j in range(EC):
            pt = pieces.tile([P, 512], F32, tag="piece", name=f"ptE{i}{j}")
            nc.tensor.transpose(out=pt[:, :P],
                                in_=enc_sb[:, i, j * P:(j + 1) * P],
                                identity=ident[:])
            nc.vector.tensor_copy(encT16[:, j, i * P:(i + 1) * P], pt[:, :P])

    # ---- UhT[e,y] then WsT[e,x] (fp16 matmuls into PSUM) ----
    UhT_ps = psA.tile([P, EC, Ty], F32)
    WsT_ps = psA.tile([P, EC, Tx], F32)
    for co in range(EC):
        for ci in range(DC):
            nc.tensor.matmul(
                out=UhT_ps[:, co, :],
                lhsT=U16[:, ci, co * P:(co + 1) * P],
                rhs=decT16[:, ci, :],
                start=(ci == 0), stop=(ci == DC - 1))
    for co in range(EC):
        for ci in range(EC):
            nc.tensor.matmul(
                out=WsT_ps[:, co, :],
                lhsT=W16[:, ci, co * P:(co + 1) * P],
                rhs=encT16[:, ci, :],
                start=(ci == 0), stop=(ci == EC - 1))

    # ---- combined factor tiles ----
    # fam[m]: [P, side(0=W,1=U), fam(0=sin,1=cos), ec, 256] fp16.
    # W side is V-seeded; U side unscaled. famUs[m] = c_m * fam[m][U side].
    fam = [None] * (M_MODES + 1)
    famUs = [None] * (M_MODES + 1)
    for m in range(1, M_MODES + 1):
        fam[m] = consts.tile([P, 2, 2, EC, 256], F16, name=f"fam{m}")
        famUs[m] = consts.tile([P, 2, EC, 256], F16, name=f"famUs{m}")
    # half-size multiplier [t2cW | t2cU]; the mode-loop mult reads it via a
    # step-0 broadcast AP as [t2cW,t2cW,t2cU,t2cU] (3 free dims stays on the
    # DVE fast path)
    t2half = consts.tile([P, 2, EC * 256], F16)
    t2bcast = bass.AP(tensor=t2half.tensor, offset=t2half.offset,
                      ap=[t2half.ap[0], [EC * 256, 2], [0, 2], [1, EC * 256]])
    sinW_raw = consts.tile([P, EC, Tx], F16)
    cosW_raw = consts.tile([P, EC, Tx], F16)

    # ---- full-angle trig bases on ACT: with w = pi/7.7 and |z| <= 3.82,
    # |w z| <= 1.56 and |pi/2 - w z| <= 3.13 < pi, so ACT Sin emits
    # sin(wz)/cos(wz) directly -- the U-side ones straight into the mode-1
    # family slots (no DVE products at all on the U side) ----
    nc.scalar.activation(out=_flat(fam[1][:, 1, 0]), in_=_flat(UhT_ps[:]),
                         func=SIN, scale=OMEGA)
    nc.scalar.activation(out=_flat(fam[1][:, 1, 1]), in_=_flat(UhT_ps[:]),
                         func=SIN, scale=-OMEGA, bias=halfpi_sb[:])
    nc.scalar.activation(out=_flat(sinW_raw[:]), in_=_flat(WsT_ps[:]),
                         func=SIN, scale=OMEGA)
    nc.scalar.activation(out=_flat(cosW_raw[:]), in_=_flat(WsT_ps[:]),
                         func=SIN, scale=-OMEGA, bias=halfpi_sb[:])
    nc.scalar.mul(out=_flat(famUs[1][:]), in_=_flat(fam[1][:, 1]),
                  mul=float(COEF[0]))
    # t2c = 2 cos(wz); W-side mode-1 families are V-scaled copies
    nc.vector.tensor_scalar_mul(out=_flat(t2half[:, 1, :]),
                                in0=_flat(fam[1][:, 1, 1]), scalar1=2.0)
    nc.vector.tensor_scalar_mul(out=_flat(t2half[:, 0, :]),
                                in0=_flat(cosW_raw[:]), scalar1=2.0)
    for ec in range(EC):
        nc.vector.tensor_scalar_mul(
            out=_flat(fam[1][:, 0, 0, ec, :]), in0=sinW_raw[:, ec, :],
            scalar1=V_sb[:, ec:ec + 1])
        nc.vector.tensor_scalar_mul(
            out=_flat(fam[1][:, 0, 1, ec, :]), in0=cosW_raw[:, ec, :],
            scalar1=V_sb[:, ec:ec + 1])

    def emit_mode_matmuls(m):
        for xh in range(XC):
            for f in range(2):
                for ec in range(EC):
                    nc.tensor.matmul(
                        out=logit_ps[:, xh, :],
                        lhsT=fam[m][:, 0, f, ec, xh * P:(xh + 1) * P],
                        rhs=famUs[m][:, 1 - f, ec, :],
                        start=False, stop=False,
                        skip_group_check=True)

    emit_mode_matmuls(1)

    # ---- Chebyshev recurrence per mode + PE accumulation ----
    for m in range(2, M_MODES + 1):
        if m == 2:
            # "mode 0" is [0, V | 0, 1]: the sin halves subtract zero, so
            # the mult writes fam[2] directly and only the cos halves get
            # corrected in place (per-partition V / immediate 1, on the 4x
            # tensor_scalar path)
            nc.vector.tensor_tensor(out=_flat(fam[2][:]),
                                    in0=_flat(fam[1][:]),
                                    in1=t2bcast, op=MULT)
            for ec in range(EC):
                nc.vector.tensor_scalar(
                    out=fam[2][:, 0, 1, ec, :], in0=fam[2][:, 0, 1, ec, :],
                    scalar1=V_sb[:, ec:ec + 1], scalar2=None, op0=SUB)
            nc.vector.tensor_scalar(
                out=_flat(fam[2][:, 1, 1]), in0=_flat(fam[2][:, 1, 1]),
                scalar1=1.0, scalar2=None, op0=SUB)
        else:
            tmp = tmps.tile([P, 2, 2, EC, 256], F16, tag="tmp",
                            name=f"tmp{m}")
            nc.vector.tensor_tensor(out=_flat(tmp[:]),
                                    in0=_flat(fam[m - 1][:]),
                                    in1=t2bcast, op=MULT)
            nc.vector.tensor_tensor(out=_flat(fam[m][:]), in0=_flat(tmp[:]),
                                    in1=_flat(fam[m - 2][:]), op=SUB)
        if m == M_MODES:
            # last mode's scale on DVE, split per family so the first half
            # of the mode-M matmuls (f=1 reads the sin part) starts sooner;
            # ACT prefetched the Exp table during mode M-1
            nc.vector.tensor_scalar_mul(out=_flat(famUs[m][:, 0]),
                                        in0=_flat(fam[m][:, 1, 0]),
                                        scalar1=float(COEF[m - 1]))
            for xh in range(XC):
                for ec in range(EC):
                    nc.tensor.matmul(
                        out=logit_ps[:, xh, :],
                        lhsT=fam[m][:, 0, 1, ec, xh * P:(xh + 1) * P],
                        rhs=famUs[m][:, 0, ec, :],
                        start=False, stop=False, skip_group_check=True)
            nc.vector.tensor_scalar_mul(out=_flat(famUs[m][:, 1]),
                                        in0=_flat(fam[m][:, 1, 1]),
                                        scalar1=float(COEF[m - 1]))
            for xh in range(XC):
                for ec in range(EC):
                    nc.tensor.matmul(
                        out=logit_ps[:, xh, :],
                        lhsT=fam[m][:, 0, 0, ec, xh * P:(xh + 1) * P],
                        rhs=famUs[m][:, 1, ec, :],
                        start=False, stop=False, skip_group_check=True)
            continue
        else:
            # c_m scaling on ACT (Copy with scale), parallel to the DVE
            # recurrence
            nc.scalar.mul(out=_flat(famUs[m][:]), in_=_flat(fam[m][:, 1]),
                          mul=float(COEF[m - 1]))
            if m == M_MODES - 1:
                # prefetch the Exp table during mode M; the input dep on
                # fam[m] pins this op late (the scheduler would otherwise
                # hoist it into the prologue, thrashing the Sin table)
                nc.scalar.activation(out=warm_sb[:],
                                     in_=fam[m][:, 0, 0, 0, 0:1],
                                     func=EXP)
        emit_mode_matmuls(m)

    # ---- softmax epilogue (transposed layout) ----
    expT = consts.tile([P, XC, Ty], F16)
    nc.scalar.activation(out=expT[:], in_=logit_ps[:], func=EXP)
    recip_sb = consts.tile([P, YC], F32)
    c_sb = consts.tile([P, YC, E], F32)
    alpha_sb = consts.tile([P, YC, Tx], F32)
    for yh in range(YC):
        den = pieces.tile([P, 512], F32, tag="piece", name=f"den{yh}")
        for xh in range(XC):
            nc.tensor.matmul(out=den[:, :1],
                             lhsT=expT[:, xh, yh * P:(yh + 1) * P],
                             rhs=ones16[:],
                             start=(xh == 0), stop=(xh == XC - 1))
        nc.vector.reciprocal(recip_sb[:, yh:yh + 1], den[:, :1])
        cps = pieces.tile([P, 512], F32, tag="piece", name=f"cps{yh}")
        for xh in range(XC):
            nc.tensor.matmul(out=cps[:, :E],
                             lhsT=expT[:, xh, yh * P:(yh + 1) * P],
                             rhs=enc16[:, xh, :],
                             start=(xh == 0), stop=(xh == XC - 1))
        nc.vector.tensor_scalar_mul(out=c_sb[:, yh, :], in0=cps[:, :E],
                                    scalar1=recip_sb[:, yh:yh + 1])
        nc.scalar.dma_start(out=c_d[yh * P:(yh + 1) * P, :],
                            in_=c_sb[:, yh, :])
        for xh in range(XC):
            pa = pieces.tile([P, 512], F16, tag="piece", name=f"pa{yh}{xh}")
            nc.tensor.transpose(out=pa[:, :P],
                                in_=expT[:, xh, yh * P:(yh + 1) * P],
                                identity=ident16[:])
            # scale on ACT (idle after exp) so DVE only handles the c path
            nc.scalar.activation(
                out=alpha_sb[:, yh, xh * P:(xh + 1) * P], in_=pa[:, :P],
                func=mybir.ActivationFunctionType.Identity,
                scale=recip_sb[:, yh:yh + 1])
        nc.sync.dma_start(out=e_d[yh * P:(yh + 1) * P, :],
                          in_=alpha_sb[:, yh, :])


def _build():
    nc = bacc.Bacc("TRN2", target_bir_lowering=False, debug=False,
                   num_devices=NCORES)
    enc_d = nc.dram_tensor("enc", [Tx, E], F32, kind="ExternalInput").ap()
    dec_d = nc.dram_tensor("dec", [Ty, D], F32, kind="ExternalInput").ap()
    W_d = nc.dram_tensor("W", [E, E], F32, kind="ExternalInput").ap()
    U_d = nc.dram_tensor("U", [D, E], F32, kind="ExternalInput").ap()
    V_d = nc.dram_tensor("V", [E, 1], F32, kind="ExternalInput").ap()
    c_d = nc.dram_tensor("c_out", [Ty, E], F32, kind="ExternalOutput").ap()
    e_d = nc.dram_tensor("e_out", [Ty, Tx], F32, kind="ExternalOutput").ap()

    with tile.TileContext(nc) as tc:
        with ExitStack() as ctx:
            _build_body(tc, ctx, enc_d, dec_d, W_d, U_d, V_d, c_d, e_d)
    nc.compile()
    return nc


def _get_nc():
    global _NC
    if _NC is None:
        _NC = _build()
    return _NC


def kernel(encoder_out_seq, decoder_out_seq, W_a, U_a, V_a):
    enc = np.ascontiguousarray(np.asarray(encoder_out_seq, dtype=np.float32))
    dec = np.ascontiguousarray(np.asarray(decoder_out_seq, dtype=np.float32))
    W = np.ascontiguousarray(np.asarray(W_a, dtype=np.float32))
    U = np.ascontiguousarray(np.asarray(U_a, dtype=np.float32))
    V = np.ascontiguousarray(np.asarray(V_a, dtype=np.float32))

    nc = _get_nc()
    in_maps = [
        {"enc": enc[i], "dec": dec[i], "W": W, "U": U, "V": V}
        for i in range(NCORES)
    ]
    res = run_bass_kernel_spmd(nc, in_maps, list(range(NCORES)))
    global LAST_RESULTS
    LAST_RESULTS = res
    c = np.stack([res.results[i]["c_out"] for i in range(NCORES)])
    e = np.stack([res.results[i]["e_out"] for i in range(NCORES)])
    return c, e



# revision 7
# speedup vs baseline: 1.0785x; 1.0785x over previous
"""Bahdanau additive attention on TRN2 — separable-Fourier Bass/Tile kernel, v2.

Problem: nn_AttentionLayer_11055245820581
  e[b,y,x] = softmax_x( sum_e V[e] * tanh(Ws[b,x,e] + Uh[b,y,e]) )
  c[b,y,:] = sum_x e[b,y,x] * enc[b,x,:]
with Ws = enc @ W_a, Uh = dec @ U_a.

Sharding: data-parallel over batch B=8 across the 8 NeuronCores.

tanh(z) ~= sum_{m=1..M} c_m sin(m*w*z) on |z| <= 7.7 (w = pi/7.7), and
sin(mw(a+b)) = sin_m(a)cos_m(b) + cos_m(a)sin_m(b) turns the V-weighted
e-contraction into 2M rank-E fp16 PE matmuls.

v2 over the v1 kernel (41.8us): the factor families advance by
STRIDE-2 dual Chebyshev chains with multiplier t2c2 = 2cos(2wz):
  odd chain : fam3 = (t2c2 +- 1) . fam1,  fam5 = t2c2.fam3 - fam1,
              fam7 = t2c2.fam5 - fam3
  even chain: fam2' (= fam2/2) from s1*c1 / Square(c1),
              fam4' = t2c2.fam2' - fam0', fam6' = t2c2.fam4' - fam2'
The halved even chain folds into the per-mode coefficients (gamma = 4c_m
for even m). fam4/5 and fam6/7 pairs are computed by merged [128,4096]
DVE tensor_tensor ops (pair-adjacent tiles), so the whole recurrence is
~7 big-TT-equivalents instead of 11, and the t2c2 multiplier comes from
ONE ACT Square + tensor_scalar instead of per-mode ACT work.  Mode-2 sin
bases come from a single DVE TT (s1*c1, both sides at once).  Per-mode
c_m scalings moved from ACT Copy (1377ns each) to DVE tensor_scalar @4x
(~330ns).  UhT/WsT live in ONE adjacent PSUM tile so each trig base is
a single merged [128,1024] ACT Sin over both sides.  U16 cast moved off
gpsimd (Q7-emulated, stalls DVE) to idle prologue DVE.  Epilogue: mode-M
matmuls emitted xh-major + EXP split per xh half; alpha scales split
DVE/ACT; dec/enc input DMA split per half for earlier PE transposes.

HW-measured pitfalls baked in (TRN2):
 - multi-free-dim DVE APs: only t2bcast-class shapes (0-stride lead dims,
   contiguous 512+ inner runs) are used; everything else is flat.
 - explicit start/stop accumulation bits across interleaved PSUM groups
   misassociate: memset once + start=False/stop=False everywhere.
 - ACT table loads cost ~1.5us; Sin and Exp live in different sets; the
   swap is placed after the last Sin/Square use via a pinned dummy Exp.
 - gpsimd only does dma/memset/tensor_copy of the identity.
"""

import os

os.environ.setdefault("NEURON_RT_RESET_CORES", "1")

import numpy as np
from contextlib import ExitStack

import concourse.bass as bass
import concourse.bacc as bacc
import concourse.tile as tile
from concourse import mybir
from concourse.bass_utils import run_bass_kernel_spmd

B, Tx, Ty, E, D = 8, 256, 256, 256, 256
P = 128
NCORES = 8
F32 = mybir.dt.float32
F16 = mybir.dt.float16
SIN = mybir.ActivationFunctionType.Sin
EXP = mybir.ActivationFunctionType.Exp
SQUARE = mybir.ActivationFunctionType.Square
IDENT = mybir.ActivationFunctionType.Identity
MULT = mybir.AluOpType.mult
ADD = mybir.AluOpType.add
SUB = mybir.AluOpType.subtract

EC = E // P      # 2 e-chunks
XC = Tx // P     # 2 x-chunks
YC = Ty // P     # 2 y-halves
DC = D // P      # 2 d-chunks

# Sine-series fit of tanh on |z|<=7.7 (Gaussian-weighted LSQ, wstd=3.2,
# floor=5e-3), coefficients refit for the v2 fp16 chain numerics:
# end-to-end rel err 4.2e-3 (numpy mirror of this kernel).
M_MODES = 7
L_PER = 7.7
OMEGA = float(np.pi / L_PER)
COEF = [1.227222613856828, -0.06362063635995319, 0.3190074912395547,
        -0.07340173334525109, 0.12816602876155422, -0.043337027089728836,
        0.039055147705349964]

_NC = None
LAST_RESULTS = None


def _mk(t, off, dims):
    """Manual flat AP into a tile at element offset `off`;
    dims = [(stride, count), ...] free dims."""
    return bass.AP(tensor=t.tensor, offset=t.offset + off,
                   ap=[t.ap[0]] + [[s, c] for (s, c) in dims])


def _build_body(tc, ctx, enc_d, dec_d, W_d, U_d, V_d, c_d, e_d):
    nc = tc.nc
    from concourse.masks import make_identity

    consts = ctx.enter_context(tc.tile_pool(name="consts", bufs=1))
    psA = ctx.enter_context(tc.tile_pool(name="psA", bufs=1, space="PSUM"))
    pieces = ctx.enter_context(tc.tile_pool(name="pieces", bufs=4,
                                            space="PSUM"))

    # ---- identity first: gates all PE transposes ----
    ident = consts.tile([P, P], F32)
    make_identity(nc, ident)
    ident16 = consts.tile([P, P], F16)
    nc.gpsimd.tensor_copy(ident16[:], ident[:])

    # ---- staging tiles ----
    dec_sb = consts.tile([P, YC, D], F32)      # [y-part, yh, d]
    enc_sb = consts.tile([P, XC, E], F32)      # [x-part, xh, e]
    U_sb = consts.tile([P, DC, E], F32)        # [d-part, dc, e]
    W_sb = consts.tile([P, EC, E], F32)        # [e'-part, ec, e]
    V_sb = consts.tile([P, EC], F32)

    # ---- input DMA, split halves for earlier transposes ----
    for yh in range(YC):
        nc.sync.dma_start(out=dec_sb[:, yh, :],
                          in_=dec_d[yh * P:(yh + 1) * P, :])
    nc.scalar.dma_start(out=U_sb[:],
                        in_=U_d.rearrange("(c p) e -> p c e", c=DC))
    for xh in range(XC):
        nc.gpsimd.dma_start(out=enc_sb[:, xh, :],
                            in_=enc_d[xh * P:(xh + 1) * P, :])
    nc.scalar.dma_start(out=W_sb[:],
                        in_=W_d.rearrange("(c p) e -> p c e", c=EC))
    nc.sync.dma_start(out=V_sb[:],
                      in_=V_d.rearrange("(c p) o -> p (c o)", c=EC))

    # ---- warmups: Sin table load + PE clock ramp (no input deps) ----
    halfpi_sb = consts.tile([P, 1], F32)
    nc.vector.memset(halfpi_sb[:], float(np.pi / 2))
    warm_sb = consts.tile([P, 1], F32)
    nc.scalar.activation(out=warm_sb[:], in_=halfpi_sb[:], func=SIN,
                         scale=0.1)
    pe_warm = consts.tile([P, 256], F16)
    nc.vector.memset(pe_warm[:], 1.0)
    for r in range(4):
        warm_ps = pieces.tile([P, 512], F32, tag="piece", name=f"warm{r}")
        nc.tensor.matmul(out=warm_ps[:, :256], lhsT=pe_warm[:, :P],
                         rhs=pe_warm[:], start=True, stop=True,
                         skip_group_check=True)

    ones16 = consts.tile([P, 1], F16)
    nc.vector.memset(ones16[:], 1.0)

    # logit accumulator [x-part, xh, y]: zeroed once, matmuls accumulate
    # with start=False/stop=False.
    logit_ps = psA.tile([P, XC, Ty], F32)
    nc.vector.memset(logit_ps[:], 0.0)

    # UhT and WsT in ONE psum tile: [e-part, side(0=U,1=W), co, 256]
    psAB = psA.tile([P, 2, EC, 256], F32)

    # ---- fp16 casts ----
    U16 = consts.tile([P, DC, E], F16)
    W16 = consts.tile([P, EC, E], F16)
    enc16 = consts.tile([P, XC, E], F16)
    nc.vector.tensor_copy(_mk(U16, 0, [(1, 512)]),
                          _mk(U_sb, 0, [(1, 512)]))  # DVE, early idle window
    nc.scalar.copy(_mk(W16, 0, [(1, 512)]), _mk(W_sb, 0, [(1, 512)]))

    V2_sb = consts.tile([P, EC], F32)
    nc.vector.tensor_scalar_mul(out=V2_sb[:], in0=V_sb[:], scalar1=0.5)

    # ---- fp32 PE transposes; evacs cast to fp16 on DVE ----
    decT16 = consts.tile([P, DC, Ty], F16)  # [d-part, dc, y]
    encT16 = consts.tile([P, EC, Tx], F16)  # [e'-part, ec, x]
    tps = []
    for i in range(YC):
        for j in range(DC):
            pt = pieces.tile([P, 512], F32, tag="piece", name=f"ptD{i}{j}")
            nc.tensor.transpose(out=pt[:, :P],
                                in_=dec_sb[:, i, j * P:(j + 1) * P],
                                identity=ident[:])
            tps.append((pt, decT16, j, i))
    for i in range(XC):
        for j in range(EC):
            pt = pieces.tile([P, 512], F32, tag="piece", name=f"ptE{i}{j}")
            nc.tensor.transpose(out=pt[:, :P],
                                in_=enc_sb[:, i, j * P:(j + 1) * P],
                                identity=ident[:])
            tps.append((pt, encT16, j, i))
    for pt, dst, j, i in tps:
        nc.vector.tensor_copy(dst[:, j, i * P:(i + 1) * P], pt[:, :P])

    # ---- UhT[e,y] then WsT[e,x] fp16 matmuls into the shared psum tile ----
    for co in range(EC):
        for ci in range(DC):
            nc.tensor.matmul(
                out=psAB[:, 0, co, :],
                lhsT=U16[:, ci, co * P:(co + 1) * P],
                rhs=decT16[:, ci, :],
                start=(ci == 0), stop=(ci == DC - 1))
    for co in range(EC):
        for ci in range(EC):
            nc.tensor.matmul(
                out=psAB[:, 1, co, :],
                lhsT=W16[:, ci, co * P:(co + 1) * P],
                rhs=encT16[:, ci, :],
                start=(ci == 0), stop=(ci == EC - 1))

    # ---- factor tiles ----
    # fam1raw [P,3072]: sU(0) sW_V(512) cU(1024) cW_V(1536) sWr(2048) cWr(2560)
    # chain layout within a fam: [fam(2: sin,cos), side(2: U,W), ec, 256]
    fam1 = consts.tile([P, 3072], F16)
    famA = consts.tile([P, 4096], F16)   # [fam2' | fam3]
    famB = consts.tile([P, 4096], F16)   # [fam4' | fam5]
    famC = consts.tile([P, 4096], F16)   # [fam6' | fam7]
    t2sq = consts.tile([P, 1024], F16)   # [side, ec, 256] = cos1^2
    t2c2 = consts.tile([P, 1024], F16)   # 2cos(2wz) = 4 t2sq - 2
    m3m = consts.tile([P, 2048], F16)    # [fam, side, ec, 256]: t2c2 +- 1
    famUs = [None] * (M_MODES + 1)
    for m in range(1, M_MODES + 1):
        famUs[m] = consts.tile([P, 1024], F16, name=f"famUs{m}")

    # ---- merged trig bases: ONE Sin / ONE Cos over [UhT|WsT] ----
    # in: psAB rows [U(512) | W(512)]; out rows into fam1:
    #   sin -> sU(0), sWr(2048) ; cos -> cU(1024), cWr(2560)
    ps_in = _mk(psAB, 0, [(512, 2), (1, 512)])
    nc.scalar.activation(out=_mk(fam1, 0, [(2048, 2), (1, 512)]),
                         in_=ps_in, func=SIN, scale=OMEGA)
    nc.scalar.activation(out=_mk(fam1, 1024, [(1536, 2), (1, 512)]),
                         in_=ps_in, func=SIN, scale=-OMEGA,
                         bias=halfpi_sb[:])

    # t2sq = cos1^2 (raw) per side, on ACT (Square is in every table set)
    nc.scalar.activation(out=_mk(t2sq, 0, [(1, 512)]),
                         in_=_mk(fam1, 1024, [(1, 512)]), func=SQUARE)
    nc.scalar.activation(out=_mk(t2sq, 512, [(1, 512)]),
                         in_=_mk(fam1, 2560, [(1, 512)]), func=SQUARE)
    # Exp-table prefetch: pinned after the squares (input dep on t2sq).
    nc.scalar.activation(out=warm_sb[:], in_=_mk(t2sq, 0, [(1, 1)]),
                         func=EXP)
    nc.scalar.copy(_mk(enc16, 0, [(1, 512)]),
                   _mk(enc_sb, 0, [(1, 512)]))    # ACT, mid-kernel slack

    # ---- DVE factor chain ----
    # V-scales: fam1 W slots = V * raw W bases
    for ec in range(EC):
        nc.vector.tensor_scalar_mul(
            out=_mk(fam1, 512 + ec * 256, [(1, 256)]),
            in0=_mk(fam1, 2048 + ec * 256, [(1, 256)]),
            scalar1=V_sb[:, ec:ec + 1])
    for ec in range(EC):
        nc.vector.tensor_scalar_mul(
            out=_mk(fam1, 1536 + ec * 256, [(1, 256)]),
            in0=_mk(fam1, 2560 + ec * 256, [(1, 256)]),
            scalar1=V_sb[:, ec:ec + 1])
    # s2' = s1*c1 (raw, both sides): rows [sU|sWr] x [cU|cWr]
    nc.vector.tensor_tensor(
        out=_mk(famA, 0, [(512, 2), (1, 512)]),
        in0=_mk(fam1, 0, [(2048, 2), (1, 512)]),
        in1=_mk(fam1, 1024, [(1536, 2), (1, 512)]), op=MULT)
    # V-scale the W sin slots of fam2' in place
    for ec in range(EC):
        nc.vector.tensor_scalar_mul(
            out=_mk(famA, 512 + ec * 256, [(1, 256)]),
            in0=_mk(famA, 512 + ec * 256, [(1, 256)]),
            scalar1=V_sb[:, ec:ec + 1])
    # famUs1 = c1 * [sU | cU]
    nc.vector.tensor_scalar_mul(
        out=_mk(famUs[1], 0, [(512, 2), (1, 512)]),
        in0=_mk(fam1, 0, [(1024, 2), (1, 512)]),
        scalar1=float(COEF[0]))
    # t2c2 = 4 t2sq - 2 ; m3 sin = 4 t2sq - 1 ; m3 cos = 4 t2sq - 3
    nc.vector.tensor_scalar(out=t2c2[:], in0=t2sq[:], scalar1=4.0,
                            scalar2=2.0, op0=MULT, op1=SUB)
    nc.vector.tensor_scalar(out=_mk(m3m, 0, [(1, 1024)]), in0=t2sq[:],
                            scalar1=4.0, scalar2=1.0, op0=MULT, op1=SUB)
    nc.vector.tensor_scalar(out=_mk(m3m, 1024, [(1, 1024)]), in0=t2sq[:],
                            scalar1=4.0, scalar2=3.0, op0=MULT, op1=SUB)
    # fam2' cos slots: U = t2sq - 0.5 ; W = (t2sq - 0.5) * V
    nc.vector.tensor_scalar(out=_mk(famA, 1024, [(1, 512)]),
                            in0=_mk(t2sq, 0, [(1, 512)]),
                            scalar1=0.5, scalar2=None, op0=SUB)
    for ec in range(EC):
        nc.vector.tensor_scalar(
            out=_mk(famA, 1536 + ec * 256, [(1, 256)]),
            in0=_mk(t2sq, 512 + ec * 256, [(1, 256)]),
            scalar1=0.5, scalar2=V_sb[:, ec:ec + 1], op0=SUB, op1=MULT)
    # famUs2 = 4 c2 * fam2'[U]
    nc.vector.tensor_scalar_mul(
        out=_mk(famUs[2], 0, [(512, 2), (1, 512)]),
        in0=_mk(famA, 0, [(1024, 2), (1, 512)]),
        scalar1=float(4.0 * COEF[1]))
    # fam3 = m3 . fam1
    nc.vector.tensor_tensor(out=_mk(famA, 2048, [(1, 2048)]),
                            in0=_mk(fam1, 0, [(1, 2048)]),
                            in1=_mk(m3m, 0, [(1, 2048)]), op=MULT)
    nc.vector.tensor_scalar_mul(
        out=_mk(famUs[3], 0, [(512, 2), (1, 512)]),
        in0=_mk(famA, 2048, [(1024, 2), (1, 512)]),
        scalar1=float(COEF[2]))
    # famB = t2c2 . famA  (fam4' and fam5 mults in one [128,4096] TT)
    t2c2_b4 = _mk(t2c2, 0, [(0, 2), (0, 2), (1, 1024)])
    sh4096 = [(2048, 2), (1024, 2), (1, 1024)]
    nc.vector.tensor_tensor(out=_mk(famB, 0, sh4096),
                            in0=_mk(famA, 0, sh4096),
                            in1=t2c2_b4, op=MULT)
    # fam5 -= fam1
    nc.vector.tensor_tensor(out=_mk(famB, 2048, [(1, 2048)]),
                            in0=_mk(famB, 2048, [(1, 2048)]),
                            in1=_mk(fam1, 0, [(1, 2048)]), op=SUB)
    # fam4' cos fixes: U -= 0.5 ; W -= 0.5V
    nc.vector.tensor_scalar(out=_mk(famB, 1024, [(1, 512)]),
                            in0=_mk(famB, 1024, [(1, 512)]),
                            scalar1=0.5, scalar2=None, op0=SUB)
    for ec in range(EC):
        nc.vector.tensor_scalar(
            out=_mk(famB, 1536 + ec * 256, [(1, 256)]),
            in0=_mk(famB, 1536 + ec * 256, [(1, 256)]),
            scalar1=V2_sb[:, ec:ec + 1], scalar2=None, op0=SUB)
    nc.vector.tensor_scalar_mul(
        out=_mk(famUs[4], 0, [(512, 2), (1, 512)]),
        in0=_mk(famB, 0, [(1024, 2), (1, 512)]),
        scalar1=float(4.0 * COEF[3]))
    nc.vector.tensor_scalar_mul(
        out=_mk(famUs[5], 0, [(512, 2), (1, 512)]),
        in0=_mk(famB, 2048, [(1024, 2), (1, 512)]),
        scalar1=float(COEF[4]))
    if M_MODES >= 7:
        # famC = t2c2 . famB - famA  (fam6' and fam7 in two [128,4096] TTs)
        nc.vector.tensor_tensor(out=_mk(famC, 0, sh4096),
                                in0=_mk(famB, 0, sh4096),
                                in1=t2c2_b4, op=MULT)
        nc.vector.tensor_tensor(out=_mk(famC, 0, [(1, 4096)]),
                                in0=_mk(famC, 0, [(1, 4096)]),
                                in1=_mk(famA, 0, [(1, 4096)]), op=SUB)
    else:
        # M=6: only fam6' = t2c2 . fam4' - fam2'
        sh2048 = [(1024, 2), (1, 1024)]
        nc.vector.tensor_tensor(out=_mk(famC, 0, sh2048),
                                in0=_mk(famB, 0, sh2048),
                                in1=_mk(t2c2, 0, [(0, 2), (1, 1024)]),
                                op=MULT)
        nc.vector.tensor_tensor(out=_mk(famC, 0, [(1, 2048)]),
                                in0=_mk(famC, 0, [(1, 2048)]),
                                in1=_mk(famA, 0, [(1, 2048)]), op=SUB)
    nc.vector.tensor_scalar_mul(
        out=_mk(famUs[6], 0, [(512, 2), (1, 512)]),
        in0=_mk(famC, 0, [(1024, 2), (1, 512)]),
        scalar1=float(4.0 * COEF[5]))
    if M_MODES >= 7:
        nc.vector.tensor_scalar_mul(
            out=_mk(famUs[7], 0, [(512, 2), (1, 512)]),
            in0=_mk(famC, 2048, [(1024, 2), (1, 512)]),
            scalar1=float(COEF[6]))

    # ---- logit matmuls: logitT[x,y] += famW_m(f)^T famUs_m(1-f) ----
    fam_base = {1: (fam1, 0), 2: (famA, 0), 3: (famA, 2048),
                4: (famB, 0), 5: (famB, 2048), 6: (famC, 0),
                7: (famC, 2048)}
    # fam1 W slots live at sW_V(512)/cW_V(1536) inside fam1 (stride 1024
    # between sin and cos halves, same as the chain tiles).
    def lhsT_ap(m, f, ec, xh):
        t, base = fam_base[m]
        return _mk(t, base + f * 1024 + 512 + ec * 256 + xh * P, [(1, P)])

    def emit_mode(m, xhs):
        for xh in xhs:
            for f in range(2):
                for ec in range(EC):
                    nc.tensor.matmul(
                        out=logit_ps[:, xh, :],
                        lhsT=lhsT_ap(m, f, ec, xh),
                        rhs=_mk(famUs[m], (1 - f) * 512 + ec * 256,
                                [(1, 256)]),
                        start=False, stop=False, skip_group_check=True)

    for m in range(1, M_MODES):
        emit_mode(m, range(XC))
    # last mode xh-major, EXP per xh half right behind it
    expT = consts.tile([P, XC, Ty], F16)
    emit_mode(M_MODES, [0])
    nc.scalar.activation(out=expT[:, 0, :], in_=logit_ps[:, 0, :], func=EXP)
    emit_mode(M_MODES, [1])
    nc.scalar.activation(out=expT[:, 1, :], in_=logit_ps[:, 1, :], func=EXP)

    # ---- softmax epilogue ----
    recip_sb = consts.tile([P, YC], F32)
    c_sb = consts.tile([P, YC, E], F32)
    alpha_sb = consts.tile([P, YC, Tx], F32)
    for yh in range(YC):
        den = pieces.tile([P, 512], F32, tag="piece", name=f"den{yh}")
        for xh in range(XC):
            nc.tensor.matmul(out=den[:, :1],
                             lhsT=expT[:, xh, yh * P:(yh + 1) * P],
                             rhs=ones16[:],
                             start=(xh == 0), stop=(xh == XC - 1))
        nc.vector.reciprocal(recip_sb[:, yh:yh + 1], den[:, :1])
        cps = pieces.tile([P, 512], F32, tag="piece", name=f"cps{yh}")
        for xh in range(XC):
            nc.tensor.matmul(out=cps[:, :E],
                             lhsT=expT[:, xh, yh * P:(yh + 1) * P],
                             rhs=enc16[:, xh, :],
                             start=(xh == 0), stop=(xh == XC - 1))
        nc.vector.tensor_scalar_mul(out=c_sb[:, yh, :], in0=cps[:, :E],
                                    scalar1=recip_sb[:, yh:yh + 1])
        nc.scalar.dma_start(out=c_d[yh * P:(yh + 1) * P, :],
                            in_=c_sb[:, yh, :])
        for xh in range(XC):
            pa = pieces.tile([P, 512], F16, tag="piece", name=f"pa{yh}{xh}")
            nc.tensor.transpose(out=pa[:, :P],
                                in_=expT[:, xh, yh * P:(yh + 1) * P],
                                identity=ident16[:])
            dst = alpha_sb[:, yh, xh * P:(xh + 1) * P]
            if xh == 0:
                nc.vector.tensor_scalar_mul(
                    out=dst, in0=pa[:, :P],
                    scalar1=recip_sb[:, yh:yh + 1])
            else:
                nc.scalar.activation(out=dst, in_=pa[:, :P], func=IDENT,
                                     scale=recip_sb[:, yh:yh + 1])
            nc.sync.dma_start(
                out=e_d[yh * P:(yh + 1) * P, xh * P:(xh + 1) * P],
                in_=alpha_sb[:, yh, xh * P:(xh + 1) * P])


def _build():
    nc = bacc.Bacc("TRN2", target_bir_lowering=False, debug=False,
                   num_devices=NCORES)
    enc_d = nc.dram_tensor("enc", [Tx, E], F32, kind="ExternalInput").ap()
    dec_d = nc.dram_tensor("dec", [Ty, D], F32, kind="ExternalInput").ap()
    W_d = nc.dram_tensor("W", [E, E], F32, kind="ExternalInput").ap()
    U_d = nc.dram_tensor("U", [D, E], F32, kind="ExternalInput").ap()
    V_d = nc.dram_tensor("V", [E, 1], F32, kind="ExternalInput").ap()
    c_d = nc.dram_tensor("c_out", [Ty, E], F32, kind="ExternalOutput").ap()
    e_d = nc.dram_tensor("e_out", [Ty, Tx], F32, kind="ExternalOutput").ap()

    with tile.TileContext(nc) as tc:
        with ExitStack() as ctx:
            _build_body(tc, ctx, enc_d, dec_d, W_d, U_d, V_d, c_d, e_d)
    nc.compile()
    return nc


def _get_nc():
    global _NC
    if _NC is None:
        _NC = _build()
    return _NC


def kernel(encoder_out_seq, decoder_out_seq, W_a, U_a, V_a):
    enc = np.ascontiguousarray(np.asarray(encoder_out_seq, dtype=np.float32))
    dec = np.ascontiguousarray(np.asarray(decoder_out_seq, dtype=np.float32))
    W = np.ascontiguousarray(np.asarray(W_a, dtype=np.float32))
    U = np.ascontiguousarray(np.asarray(U_a, dtype=np.float32))
    V = np.ascontiguousarray(np.asarray(V_a, dtype=np.float32))

    nc = _get_nc()
    in_maps = [
        {"enc": enc[i], "dec": dec[i], "W": W, "U": U, "V": V}
        for i in range(NCORES)
    ]
    res = run_bass_kernel_spmd(nc, in_maps, list(range(NCORES)))
    global LAST_RESULTS
    LAST_RESULTS = res
    c = np.stack([res.results[i]["c_out"] for i in range(NCORES)])
    e = np.stack([res.results[i]["e_out"] for i in range(NCORES)])
    return c, e
